# revision 6
# baseline (speedup 1.0000x reference)
"""Multi-head attention (B=4, L=2048, D=512, H=8) on 8 Trainium2 cores.

Sharding: core c handles batch b = c//2, query rows [(c%2)*1024, +1024).
Fully local: each core projects the FULL K/V for its batch (no collectives,
no cross-core sync).

Key optimizations:
  * Score matmuls have K=dk=64, so heads 2i/2i+1 are packed into PE row
    tiles (0,0)/(64,0) and run CONCURRENTLY (2x score throughput). The
    qT/kT layout puts head 2m at partitions 0:64 and 2m+1 at 64:128 of
    dmodel-chunk m, so base_partition auto-derives the tile_position.
  * All PE instructions are chained with scheduler ordering deps so the
    emission order survives: paired score MMs stay adjacent, and 64-row /
    128-row array reconfigurations (drains) happen once per 16-MM block
    instead of every other instruction.
  * exp(scores) is split across TWO engines: ACT computes exact Exp;
    DVE computes a Schraudolph bit-trick exp (one tensor_scalar:
    i16 = A16*s + B16, int16 bits reinterpreted as bf16). Softmax
    renormalization cancels the ~3% multiplicative error.
  * Q/K bias+evacuation fused into one ACT activation (per-partition bias).
  * V/O biases folded into the matmul accumulation via K=1 ones-row MMs.

Per-core device layout:
  xqT (512,1024) xkT/xvT (512,2048)  inputs transposed (dmodel on partitions)
  qT (128,1024)x4  kT (128,2048)x4   head h at chunk h//2, partitions 64*(h%2)
  V  (128, 544)x16                   kv chunk tiles; head h cols [68h,68h+64)
                                     data, col 68h+64 = ones (denominator)
  ss (128,512) PSUM                  scores [kv, q-half] (1 bank)
  xs (65,1024) PSUM                  attnV accum; row 64 = softmax denom
"""
import numpy as np
import ml_dtypes

import concourse.bacc as bacc
import concourse.bass as bass
import concourse.mybir as mybir
import concourse.tile as tile
from concourse.bass_utils import run_bass_kernel_spmd

F32 = mybir.dt.float32
BF16 = mybir.dt.bfloat16
I16 = mybir.dt.int16
AF = mybir.ActivationFunctionType
ALU = mybir.AluOpType

B, L, D = 4, 2048, 512
H, DK = 8, 64
N_CORES = 8
LQ = L // 2            # query rows per core
P = 128
KVC = L // P           # 16 kv chunks
MC = D // P            # 4 dmodel chunks
VW = 68                # per-head stride in V tiles (64 data + ones + pad)
MASK_BIAS = np.float32(-30.0)   # large enough: exp(-30+s) ~ 0

# Schraudolph exp constants (bf16-bits variant): bf16_bits(exp(x)) ~=
# int16(A16*x + B16)
_A = 2.0 ** 23 / np.log(2.0)
_C = 486411.0
A16 = float(_A / 65536.0)
B16 = float((127.0 * 2.0 ** 23 - _C) / 65536.0)

MM_DT = BF16
MM_NP = ml_dtypes.bfloat16

_cache = {}


def _build():
    nc = bacc.Bacc("TRN2", target_bir_lowering=False, debug=False,
                   num_devices=N_CORES)

    xqT_d = nc.dram_tensor("xqT", [D, LQ], MM_DT, kind="ExternalInput").ap()
    xkT_d = nc.dram_tensor("xkT", [D, L], MM_DT, kind="ExternalInput").ap()
    xvT_d = nc.dram_tensor("xvT", [D, L], MM_DT, kind="ExternalInput").ap()
    wq_d = nc.dram_tensor("wq", [D, D], MM_DT, kind="ExternalInput").ap()
    wk_d = nc.dram_tensor("wk", [D, D], MM_DT, kind="ExternalInput").ap()
    wv_d = nc.dram_tensor("wv", [D, D], MM_DT, kind="ExternalInput").ap()
    wo_d = nc.dram_tensor("wo", [D, D], MM_DT, kind="ExternalInput").ap()
    bq_d = nc.dram_tensor("bq", [P, MC], F32, kind="ExternalInput").ap()
    bk_d = nc.dram_tensor("bk", [P, MC], F32, kind="ExternalInput").ap()
    bv_d = nc.dram_tensor("bv", [1, D], MM_DT, kind="ExternalInput").ap()
    bo_d = nc.dram_tensor("bo", [1, D], MM_DT, kind="ExternalInput").ap()
    mb_d = nc.dram_tensor("mb", [P, KVC], F32, kind="ExternalInput").ap()
    b2_d = nc.dram_tensor("b2", [P, KVC], F32, kind="ExternalInput").ap()
    out_d = nc.dram_tensor("out", [LQ, D], F32, kind="ExternalOutput").ap()

    from concourse.bass import _add_dep_helper
    pe_prev = [None]

    def mm(*args, **kwargs):
        inst = nc.tensor.matmul(*args, **kwargs)
        if pe_prev[0] is not None:
            _add_dep_helper(inst.ins, pe_prev[0].ins, sync=False,
                            reason="pe-order")
        pe_prev[0] = inst
        return inst

    with tile.TileContext(nc) as tc:
        with tc.tile_pool(name="const", bufs=1) as cpool, \
             tc.tile_pool(name="xin", bufs=1) as xpool, \
             tc.tile_pool(name="proj", bufs=1) as prpool, \
             tc.tile_pool(name="attn", bufs=16) as apool, \
             tc.tile_pool(name="norm", bufs=2) as npool, \
             tc.tile_pool(name="outp", bufs=3) as opool, \
             tc.tile_pool(name="ps", bufs=1, space="PSUM") as ps:

            def load_chunks(pool, ap2d, nm):
                out = []
                for kc in range(MC):
                    t = pool.tile([P, ap2d.shape[1]], ap2d.dtype,
                                  tag=f"{nm}{kc}", name=f"{nm}{kc}")
                    nc.sync.dma_start(t[:], ap2d[kc * P:(kc + 1) * P, :])
                    out.append(t)
                return out

            # interleave weight/input chunk loads in first-use order
            wq = load_chunks(cpool, wq_d, "wq")
            xqT = load_chunks(xpool, xqT_d, "xq")
            bq = cpool.tile_from(bq_d)
            wk = load_chunks(cpool, wk_d, "wk")
            xkT = load_chunks(xpool, xkT_d, "xk")
            bk = cpool.tile_from(bk_d)
            wv = load_chunks(cpool, wv_d, "wv")
            xvT = load_chunks(xpool, xvT_d, "xv")
            bv = cpool.tile_from(bv_d)
            wo = load_chunks(cpool, wo_d, "wo")
            bo = cpool.tile_from(bo_d)
            mb = cpool.tile_from(mb_d)
            b2 = cpool.tile_from(b2_d)
            ones1 = cpool.tile([1, P], MM_DT)
            nc.vector.memset(ones1[:], 1.0)

            pj_tag = [0]

            def proj_ps():
                # projection-phase PSUM supertiles share the "xs" tag banks
                pj_tag[0] ^= 1
                return ps.tile([P, 1024], F32, tag="xs", bufs=2,
                               name="pj")

            # ---- Q projection (8 MMs + 1 ACT bias/evac per m-chunk) ----
            qT = [prpool.tile([P, LQ], MM_DT, tag=f"qT{m}", name=f"qT{m}")
                  for m in range(MC)]
            for m in range(MC):
                pp = proj_ps()
                for s in range(LQ // 512):
                    for kc in range(MC):
                        mm(pp[:, s * 512:(s + 1) * 512],
                           wq[kc][:, m * P:(m + 1) * P],
                           xqT[kc][:, s * 512:(s + 1) * 512],
                           start=kc == 0, stop=kc == MC - 1)
                nc.scalar.activation(qT[m][:], pp[:], AF.Identity,
                                     bias=bq[:, m:m + 1])

            # ---- K projection (full batch: 2048 kv rows) ----
            kT = [prpool.tile([P, L], MM_DT, tag=f"kT{m}", name=f"kT{m}")
                  for m in range(MC)]
            for m in range(MC):
                for half in range(2):
                    pp = proj_ps()
                    for s in range(2):
                        for kc in range(MC):
                            mm(pp[:, s * 512:(s + 1) * 512],
                               wk[kc][:, m * P:(m + 1) * P],
                               xkT[kc][:, half * 1024 + s * 512:
                                        half * 1024 + (s + 1) * 512],
                               start=kc == 0, stop=kc == MC - 1)
                    nc.scalar.activation(
                        kT[m][:, half * 1024:(half + 1) * 1024], pp[:],
                        AF.Identity, bias=bk[:, m:m + 1])

            # ---- V projection (natural layout, ones col per head group) ----
            v_sb = prpool.tile([P, KVC * VW * H], MM_DT, tag="V", name="v_sb")
            v_g = v_sb.rearrange("p (t h d) -> p t h d", t=KVC, d=VW)
            nc.vector.memset(v_sb[:], 1.0)
            for tp in range(KVC // 2):
                pv = proj_ps()
                for j in range(2):
                    t = 2 * tp + j
                    for kc in range(MC):
                        mm(pv[:, j * 512:(j + 1) * 512],
                           xvT[kc][:, t * P:(t + 1) * P],
                           wv[kc][:, :], start=kc == 0, stop=False)
                    mm(pv[:, j * 512:(j + 1) * 512],
                       ones1[0:1, :], bv[0:1, :], start=False, stop=True)
                nc.vector.tensor_copy(
                    v_g[:, 2 * tp:2 * tp + 2, :, 0:64],
                    pv.rearrange("p (j h d) -> p j h d", j=2, d=64))

            def v_head(t, h):
                return v_g[:, t, h, 0:65]

            # ---- attention: head pairs (2i, 2i+1), PE row-tile packing ----
            # 4-chunk blocks: [scores c..c+3][attnV c-4..c-1] so the PE's
            # 64-row <-> 128-row reconfiguration happens 2x per 32 MMs.
            xsT2 = [prpool.tile([P, LQ], MM_DT, tag=f"xs{hp}",
                                name=f"xsT2_{hp}")
                    for hp in range(MC)]
            # (c, qh) of odd-head tiles that go to ACT anyway, for balance
            act_extra = {(3, 0), (8, 0), (13, 0)}
            BLK = 4
            for hp in range(MC):
                hE, hO = 2 * hp, 2 * hp + 1
                xsE = ps.tile([65, LQ], F32, tag="xs", bufs=2, name=f"xsE{hp}")
                xsO = ps.tile([65, LQ], F32, tag="xs", bufs=2, name=f"xsO{hp}")
                at_tiles = {}
                exps = []

                def scores(c):
                    for qh in range(2):
                        ssA = ps.tile([P, 512], F32, tag="ssA", bufs=2,
                                      name="ssA")
                        ssB = ps.tile([P, 512], F32, tag="ssB", bufs=2,
                                      name="ssB")
                        mm(ssA[:], kT[hp][0:64, c * P:(c + 1) * P],
                           qT[hp][0:64, qh * 512:(qh + 1) * 512],
                           start=True, stop=True)
                        mm(ssB[:], kT[hp][64:128, c * P:(c + 1) * P],
                           qT[hp][64:128, qh * 512:(qh + 1) * 512],
                           start=True, stop=True)
                        aE = apool.tile([P, 512], MM_DT, tag=f"atE{qh}",
                                        bufs=10, name="aE")
                        aO = apool.tile([P, 512], MM_DT, tag=f"atO{qh}",
                                        bufs=10, name="aO")
                        exps.append((ssA, ssB, aE, aO, c, qh))
                        at_tiles[(c, qh)] = (aE, aO)

                def flush_exps():
                    # emit exp ops after the whole scores block so the PE
                    # instruction chain stays contiguous in emission order
                    for ssA, ssB, aE, aO, c, qh in exps:
                        nc.scalar.activation(aE[:], ssA[:], AF.Exp,
                                             bias=mb[:, c:c + 1], scale=0.125)
                        if (c, qh) in act_extra:
                            nc.scalar.activation(aO[:], ssB[:], AF.Exp,
                                                 bias=mb[:, c:c + 1],
                                                 scale=0.125)
                        else:
                            nc.vector.tensor_scalar(
                                aO.bitcast(I16)[:], ssB[:], A16 * 0.125,
                                b2[:, c:c + 1], ALU.mult, ALU.add)
                    exps.clear()

                def attnv(c):
                    for qh in range(2):
                        aE, aO = at_tiles.pop((c, qh))
                        mm(xsE[:, qh * 512:(qh + 1) * 512], v_head(c, hE),
                           aE[:], start=c == 0, stop=c == KVC - 1,
                           skip_group_check=True)
                        mm(xsO[:, qh * 512:(qh + 1) * 512], v_head(c, hO),
                           aO[:], start=c == 0, stop=c == KVC - 1,
                           skip_group_check=True)

                for blk in range(KVC // BLK):
                    for c in range(blk * BLK, (blk + 1) * BLK):
                        scores(c)
                    flush_exps()
                    if blk:
                        for c in range((blk - 1) * BLK, blk * BLK):
                            attnv(c)
                for c in range(KVC - BLK, KVC):
                    attnv(c)

                # normalize: xsT2 = xs[0:64] / xs[64]  (denominator row)
                for par, xs_t in ((0, xsE), (1, xsO)):
                    srow = npool.tile([1, LQ], F32, tag="srow")
                    nc.scalar.copy(srow[:], xs_t[64:65, :])
                    rec = npool.tile([1, LQ], F32, tag="rec")
                    nc.vector.reciprocal_approx_fast(rec[:], srow[:])
                    bc = npool.tile([64, LQ], F32, tag="bc")
                    nc.gpsimd.partition_broadcast(bc[:], rec[:])
                    nc.vector.tensor_tensor(
                        xsT2[hp][64 * par:64 * par + 64, :],
                        xs_t[0:64, :], bc[:], ALU.mult)

            # ---- output projection (bias via K=1 ones MM, ACT evac) ----
            for qt2 in range(LQ // 256):
                po_ = proj_ps()
                for j in range(2):
                    qt = 2 * qt2 + j
                    for hp in range(MC):
                        mm(po_[:, j * 512:(j + 1) * 512],
                           xsT2[hp][:, qt * P:(qt + 1) * P],
                           wo[hp][:, :], start=hp == 0, stop=False)
                    mm(po_[:, j * 512:(j + 1) * 512],
                       ones1[0:1, 0:P], bo[0:1, :], start=False, stop=True)
                osb = opool.tile([P, 1024], F32, tag="osb")
                nc.scalar.copy(osb[:], po_[:])
                for j in range(2):
                    qt = 2 * qt2 + j
                    nc.sync.dma_start(out_d[qt * P:(qt + 1) * P, :],
                                      osb[:, j * 512:(j + 1) * 512])

    nc.compile()
    return nc


def _host_inputs(query, key, value, mask, Wq, bq, Wk, bk, Wv, bv, Wo, bo):
    """Build the 8 per-core input maps (all rank-dependence lives here)."""
    f32 = np.float32
    wq_ = np.ascontiguousarray(Wq).astype(MM_NP)
    wk_ = np.ascontiguousarray(Wk).astype(MM_NP)
    wv_ = np.ascontiguousarray(Wv).astype(MM_NP)
    wo_ = np.ascontiguousarray(Wo).astype(MM_NP)
    bq_ = np.ascontiguousarray(bq.astype(f32).reshape(MC, P).T)
    bk_ = np.ascontiguousarray(bk.astype(f32).reshape(MC, P).T)
    bv_ = bv.astype(MM_NP).reshape(1, D)
    bo_ = bo.astype(MM_NP).reshape(1, D)
    in_maps = []
    for c in range(N_CORES):
        b, half = c // 2, c % 2
        sl = slice(half * LQ, (half + 1) * LQ)
        xqT = np.ascontiguousarray(query[b, sl, :].T).astype(MM_NP)
        xkT = np.ascontiguousarray(key[b].T).astype(MM_NP)
        xvT = np.ascontiguousarray(value[b].T).astype(MM_NP)
        mbias = np.where(mask[b] == 0, MASK_BIAS, f32(0.0)).astype(f32)
        mb_ = np.ascontiguousarray(mbias.reshape(KVC, P).T)
        b2_ = (mb_ * f32(A16) + f32(B16)).astype(f32)
        in_maps.append({
            "xqT": xqT, "xkT": xkT, "xvT": xvT,
            "wq": wq_, "wk": wk_, "wv": wv_, "wo": wo_,
            "bq": bq_, "bk": bk_, "bv": bv_, "bo": bo_,
            "mb": mb_, "b2": b2_,
        })
    return in_maps


def kernel(query, key, value, mask, Wq, bq, Wk, bk, Wv, bv, Wo, bo):
    if "nc" not in _cache:
        _cache["nc"] = _build()
    nc = _cache["nc"]
    in_maps = _host_inputs(query, key, value, mask,
                           Wq, bq, Wk, bk, Wv, bv, Wo, bo)
    res = run_bass_kernel_spmd(nc, in_maps, list(range(N_CORES))).results
    out = np.empty((B, L, D), np.float32)
    for c in range(N_CORES):
        b, half = c // 2, c % 2
        out[b, half * LQ:(half + 1) * LQ, :] = res[c]["out"]
    return out


# revision 11
# speedup vs baseline: 1.4296x; 1.4296x over previous
"""Multi-head attention (B=4, L=2048, D=512, H=8) on 8 Trainium2 cores.

Sharding: core c handles batch b = c//2, query rows [(c%2)*1024, +1024).
Fully local: each core projects the FULL K/V for its batch (no collectives,
no cross-core sync).

Key optimizations:
  * EVERY matmul in the kernel runs in the PE's 128x128 array mode, so the
    array never reconfigures (mode switches drain the systolic pipe and
    cost ~160ns per matmul when score/attnV matmuls alternate). Scores
    have K=dk=64 only, so kT is stored as TWO zero-padded stationaries:
    kT_E (even head rows 0:64, zeros below) and kT_O (zeros above, odd
    head rows 64:128); the full-height qT moving operand then hits zero
    weights for the other head's rows.
  * exp(scores) is split across TWO engines: ACT computes exact Exp;
    DVE computes a Schraudolph bit-trick exp (one tensor_scalar:
    i16 = A16*s + B16, int16 bits reinterpreted as bf16). Softmax
    renormalization cancels the ~3% multiplicative error.
  * Q/K bias+evacuation fused into one ACT activation (per-partition bias).
  * V/O biases folded into the matmul accumulation via K=1 ones-row MMs.

Per-core device layout:
  xqT (512,1024) xkT/xvT (512,2048)  inputs transposed (dmodel on partitions)
  qT (128,1024)x4  kT (128,2048)x4   head h at chunk h//2, partitions 64*(h%2)
  V  (128, 544)x16                   kv chunk tiles; head h cols [68h,68h+64)
                                     data, col 68h+64 = ones (denominator)
  ss (128,512) PSUM                  scores [kv, q-half] (1 bank)
  xs (65,1024) PSUM                  attnV accum; row 64 = softmax denom
"""
import numpy as np
import ml_dtypes

import concourse.bacc as bacc
import concourse.bass as bass
import concourse.mybir as mybir
import concourse.tile as tile
from concourse.bass_utils import run_bass_kernel_spmd

F32 = mybir.dt.float32
BF16 = mybir.dt.bfloat16
I16 = mybir.dt.int16
AF = mybir.ActivationFunctionType
ALU = mybir.AluOpType

B, L, D = 4, 2048, 512
H, DK = 8, 64
N_CORES = 8
LQ = L // 2            # query rows per core
P = 128
KVC = L // P           # 16 kv chunks
MC = D // P            # 4 dmodel chunks
VW = 68                # per-head stride in V tiles (64 data + ones + pad)
MASK_BIAS = np.float32(-30.0)   # large enough: exp(-30+s) ~ 0

# Schraudolph exp constants (bf16-bits variant): bf16_bits(exp(x)) ~=
# int16(A16*x + B16)
_A = 2.0 ** 23 / np.log(2.0)
_C = 486411.0
A16 = float(_A / 65536.0)
B16 = float((127.0 * 2.0 ** 23 - _C) / 65536.0)

MM_DT = BF16
MM_NP = ml_dtypes.bfloat16

_cache = {}


def _build():
    nc = bacc.Bacc("TRN2", target_bir_lowering=False, debug=False,
                   num_devices=N_CORES)

    xqT_d = nc.dram_tensor("xqT", [D, LQ], MM_DT, kind="ExternalInput").ap()
    xkT_d = nc.dram_tensor("xkT", [D, L], MM_DT, kind="ExternalInput").ap()
    xvT_d = nc.dram_tensor("xvT", [D, L], MM_DT, kind="ExternalInput").ap()
    wq_d = nc.dram_tensor("wq", [D, D], MM_DT, kind="ExternalInput").ap()
    wk_d = nc.dram_tensor("wk", [D, D], MM_DT, kind="ExternalInput").ap()
    wv_d = nc.dram_tensor("wv", [D, D], MM_DT, kind="ExternalInput").ap()
    wo_d = nc.dram_tensor("wo", [D, D], MM_DT, kind="ExternalInput").ap()
    bq_d = nc.dram_tensor("bq", [P, MC], F32, kind="ExternalInput").ap()
    bk_d = nc.dram_tensor("bk", [P, MC], F32, kind="ExternalInput").ap()
    bv_d = nc.dram_tensor("bv", [1, D], MM_DT, kind="ExternalInput").ap()
    bo_d = nc.dram_tensor("bo", [1, D], MM_DT, kind="ExternalInput").ap()
    mb_d = nc.dram_tensor("mb", [P, KVC], F32, kind="ExternalInput").ap()
    b2_d = nc.dram_tensor("b2", [P, KVC], F32, kind="ExternalInput").ap()
    out_d = nc.dram_tensor("out", [LQ, D], F32, kind="ExternalOutput").ap()

    def mm(*args, **kwargs):
        return nc.tensor.matmul(*args, **kwargs)

    with tile.TileContext(nc) as tc:
        with tc.tile_pool(name="const", bufs=1) as cpool, \
             tc.tile_pool(name="xin", bufs=1) as xpool, \
             tc.tile_pool(name="proj", bufs=1) as prpool, \
             tc.tile_pool(name="attn", bufs=16) as apool, \
             tc.tile_pool(name="norm", bufs=2) as npool, \
             tc.tile_pool(name="outp", bufs=3) as opool, \
             tc.tile_pool(name="ps", bufs=1, space="PSUM") as ps:

            def load_chunks(pool, ap2d, nm):
                out = []
                for kc in range(MC):
                    t = pool.tile([P, ap2d.shape[1]], ap2d.dtype,
                                  tag=f"{nm}{kc}", name=f"{nm}{kc}")
                    nc.sync.dma_start(t[:], ap2d[kc * P:(kc + 1) * P, :])
                    out.append(t)
                return out

            # interleave weight/input chunk loads in first-use order
            wq = load_chunks(cpool, wq_d, "wq")
            xqT = load_chunks(xpool, xqT_d, "xq")
            bq = cpool.tile_from(bq_d)
            wk = load_chunks(cpool, wk_d, "wk")
            xkT = load_chunks(xpool, xkT_d, "xk")
            bk = cpool.tile_from(bk_d)
            wv = load_chunks(cpool, wv_d, "wv")
            xvT = load_chunks(xpool, xvT_d, "xv")
            bv = cpool.tile_from(bv_d)
            wo = load_chunks(cpool, wo_d, "wo")
            bo = cpool.tile_from(bo_d)
            mb = cpool.tile_from(mb_d)
            b2 = cpool.tile_from(b2_d)
            ones1 = cpool.tile([1, P], MM_DT)
            nc.vector.memset(ones1[:], 1.0)

            pj_tag = [0]

            def proj_ps():
                # projection-phase PSUM supertiles share the "xs" tag banks
                pj_tag[0] ^= 1
                return ps.tile([P, 1024], F32, tag="xs", bufs=2,
                               name="pj")

            # ---- Q projection (8 MMs + 1 ACT bias/evac per m-chunk) ----
            qT = [prpool.tile([P, LQ], MM_DT, tag=f"qT{m}", name=f"qT{m}")
                  for m in range(MC)]
            for m in range(MC):
                pp = proj_ps()
                for s in range(LQ // 512):
                    for kc in range(MC):
                        mm(pp[:, s * 512:(s + 1) * 512],
                           wq[kc][:, m * P:(m + 1) * P],
                           xqT[kc][:, s * 512:(s + 1) * 512],
                           start=kc == 0, stop=kc == MC - 1)
                nc.scalar.activation(qT[m][:], pp[:], AF.Identity,
                                     bias=bq[:, m:m + 1])

            # ---- K projection (full batch: 2048 kv rows) ----
            # kT is stored zero-padded per head parity so score matmuls can
            # use the full-height 128x128 array mode (no reconfiguration):
            # kT_E rows 64:128 are zero, kT_O rows 0:64 are zero.
            kT_E = [prpool.tile([P, L], MM_DT, tag=f"kTE{m}", name=f"kTE{m}")
                    for m in range(MC)]
            kT_O = [prpool.tile([P, L], MM_DT, tag=f"kTO{m}", name=f"kTO{m}")
                    for m in range(MC)]
            for m in range(MC):
                nc.vector.memset(kT_E[m][64:128, :], 0.0)
                nc.vector.memset(kT_O[m][0:64, :], 0.0)
            for m in range(MC):
                for half in range(2):
                    pp = proj_ps()
                    for s in range(2):
                        for kc in range(MC):
                            mm(pp[:, s * 512:(s + 1) * 512],
                               wk[kc][:, m * P:(m + 1) * P],
                               xkT[kc][:, half * 1024 + s * 512:
                                        half * 1024 + (s + 1) * 512],
                               start=kc == 0, stop=kc == MC - 1)
                    sl = slice(half * 1024, (half + 1) * 1024)
                    nc.scalar.activation(
                        kT_E[m][0:64, sl], pp[0:64, :],
                        AF.Identity, bias=bk[0:64, m:m + 1])
                    nc.scalar.activation(
                        kT_O[m][64:128, sl], pp[64:128, :],
                        AF.Identity, bias=bk[64:128, m:m + 1])

            # ---- V projection (natural layout, ones col per head group) ----
            v_sb = prpool.tile([P, KVC * VW * H], MM_DT, tag="V", name="v_sb")
            v_g = v_sb.rearrange("p (t h d) -> p t h d", t=KVC, d=VW)
            nc.vector.memset(v_sb[:], 1.0)
            for tp in range(KVC // 2):
                pv = proj_ps()
                for j in range(2):
                    t = 2 * tp + j
                    for kc in range(MC):
                        mm(pv[:, j * 512:(j + 1) * 512],
                           xvT[kc][:, t * P:(t + 1) * P],
                           wv[kc][:, :], start=kc == 0, stop=False)
                    mm(pv[:, j * 512:(j + 1) * 512],
                       ones1[0:1, :], bv[0:1, :], start=False, stop=True)
                nc.vector.tensor_copy(
                    v_g[:, 2 * tp:2 * tp + 2, :, 0:64],
                    pv.rearrange("p (j h d) -> p j h d", j=2, d=64))

            def v_head(t, h):
                return v_g[:, t, h, 0:65]

            # ---- attention: all matmuls in full 128x128 mode ----
            xsT2 = [prpool.tile([P, LQ], MM_DT, tag=f"xs{hp}",
                                name=f"xsT2_{hp}")
                    for hp in range(MC)]
            # (c, qh) of odd-head tiles that go to ACT anyway, for balance
            act_extra = {(3, 0), (8, 0), (13, 0)}
            for hp in range(MC):
                hE, hO = 2 * hp, 2 * hp + 1
                xsE = ps.tile([65, LQ], F32, tag="xs", bufs=2, name=f"xsE{hp}")
                xsO = ps.tile([65, LQ], F32, tag="xs", bufs=2, name=f"xsO{hp}")
                at_tiles = {}

                def scores(c):
                    for qh in range(2):
                        ssA = ps.tile([P, 512], F32, tag="ss", bufs=4,
                                      name="ssA")
                        ssB = ps.tile([P, 512], F32, tag="ss", bufs=4,
                                      name="ssB")
                        mm(ssA[:], kT_E[hp][:, c * P:(c + 1) * P],
                           qT[hp][:, qh * 512:(qh + 1) * 512],
                           start=True, stop=True)
                        mm(ssB[:], kT_O[hp][:, c * P:(c + 1) * P],
                           qT[hp][:, qh * 512:(qh + 1) * 512],
                           start=True, stop=True)
                        aE = apool.tile([P, 512], MM_DT, tag=f"atE{qh}",
                                        bufs=10, name="aE")
                        aO = apool.tile([P, 512], MM_DT, tag=f"atO{qh}",
                                        bufs=10, name="aO")
                        nc.scalar.activation(aE[:], ssA[:], AF.Exp,
                                             bias=mb[:, c:c + 1], scale=0.125)
                        if (c, qh) in act_extra:
                            nc.scalar.activation(aO[:], ssB[:], AF.Exp,
                                                 bias=mb[:, c:c + 1],
                                                 scale=0.125)
                        else:
                            nc.vector.tensor_scalar(
                                aO.bitcast(I16)[:], ssB[:], A16 * 0.125,
                                b2[:, c:c + 1], ALU.mult, ALU.add)
                        at_tiles[(c, qh)] = (aE, aO)

                def attnv(c):
                    for qh in range(2):
                        aE, aO = at_tiles.pop((c, qh))
                        mm(xsE[:, qh * 512:(qh + 1) * 512], v_head(c, hE),
                           aE[:], start=c == 0, stop=c == KVC - 1,
                           skip_group_check=True)
                        mm(xsO[:, qh * 512:(qh + 1) * 512], v_head(c, hO),
                           aO[:], start=c == 0, stop=c == KVC - 1,
                           skip_group_check=True)

                scores(0)
                scores(1)
                for c in range(2, KVC, 2):
                    scores(c)
                    scores(c + 1)
                    attnv(c - 2)
                    attnv(c - 1)
                attnv(KVC - 2)
                attnv(KVC - 1)

                # normalize: xsT2 = xs[0:64] / xs[64]  (denominator row)
                for par, xs_t in ((0, xsE), (1, xsO)):
                    srow = npool.tile([1, LQ], F32, tag="srow")
                    nc.scalar.copy(srow[:], xs_t[64:65, :])
                    rec = npool.tile([1, LQ], F32, tag="rec")
                    nc.vector.reciprocal_approx_fast(rec[:], srow[:])
                    bc = npool.tile([64, LQ], F32, tag="bc")
                    nc.gpsimd.partition_broadcast(bc[:], rec[:])
                    nc.vector.tensor_tensor(
                        xsT2[hp][64 * par:64 * par + 64, :],
                        xs_t[0:64, :], bc[:], ALU.mult)

            # ---- output projection (bias via K=1 ones MM, ACT evac) ----
            for qt2 in range(LQ // 256):
                po_ = proj_ps()
                for j in range(2):
                    qt = 2 * qt2 + j
                    for hp in range(MC):
                        mm(po_[:, j * 512:(j + 1) * 512],
                           xsT2[hp][:, qt * P:(qt + 1) * P],
                           wo[hp][:, :], start=hp == 0, stop=False)
                    mm(po_[:, j * 512:(j + 1) * 512],
                       ones1[0:1, 0:P], bo[0:1, :], start=False, stop=True)
                osb = opool.tile([P, 1024], F32, tag="osb")
                nc.scalar.copy(osb[:], po_[:])
                for j in range(2):
                    qt = 2 * qt2 + j
                    nc.sync.dma_start(out_d[qt * P:(qt + 1) * P, :],
                                      osb[:, j * 512:(j + 1) * 512])

    nc.compile()
    return nc


def _host_inputs(query, key, value, mask, Wq, bq, Wk, bk, Wv, bv, Wo, bo):
    """Build the 8 per-core input maps (all rank-dependence lives here)."""
    f32 = np.float32
    wq_ = np.ascontiguousarray(Wq).astype(MM_NP)
    wk_ = np.ascontiguousarray(Wk).astype(MM_NP)
    wv_ = np.ascontiguousarray(Wv).astype(MM_NP)
    wo_ = np.ascontiguousarray(Wo).astype(MM_NP)
    bq_ = np.ascontiguousarray(bq.astype(f32).reshape(MC, P).T)
    bk_ = np.ascontiguousarray(bk.astype(f32).reshape(MC, P).T)
    bv_ = bv.astype(MM_NP).reshape(1, D)
    bo_ = bo.astype(MM_NP).reshape(1, D)
    in_maps = []
    for c in range(N_CORES):
        b, half = c // 2, c % 2
        sl = slice(half * LQ, (half + 1) * LQ)
        xqT = np.ascontiguousarray(query[b, sl, :].T).astype(MM_NP)
        xkT = np.ascontiguousarray(key[b].T).astype(MM_NP)
        xvT = np.ascontiguousarray(value[b].T).astype(MM_NP)
        mbias = np.where(mask[b] == 0, MASK_BIAS, f32(0.0)).astype(f32)
        mb_ = np.ascontiguousarray(mbias.reshape(KVC, P).T)
        b2_ = (mb_ * f32(A16) + f32(B16)).astype(f32)
        in_maps.append({
            "xqT": xqT, "xkT": xkT, "xvT": xvT,
            "wq": wq_, "wk": wk_, "wv": wv_, "wo": wo_,
            "bq": bq_, "bk": bk_, "bv": bv_, "bo": bo_,
            "mb": mb_, "b2": b2_,
        })
    return in_maps


def kernel(query, key, value, mask, Wq, bq, Wk, bk, Wv, bv, Wo, bo):
    if "nc" not in _cache:
        _cache["nc"] = _build()
    nc = _cache["nc"]
    in_maps = _host_inputs(query, key, value, mask,
                           Wq, bq, Wk, bk, Wv, bv, Wo, bo)
    res = run_bass_kernel_spmd(nc, in_maps, list(range(N_CORES))).results
    out = np.empty((B, L, D), np.float32)
    for c in range(N_CORES):
        b, half = c // 2, c % 2
        out[b, half * LQ:(half + 1) * LQ, :] = res[c]["out"]
    return out


# revision 13
# speedup vs baseline: 1.5887x; 1.1113x over previous
"""Multi-head attention (B=4, L=2048, D=512, H=8) on 8 Trainium2 cores.

Sharding: core c handles batch b = c//2, query rows [(c%2)*1024, +1024).
Fully local: each core projects the FULL K/V for its batch (no collectives,
no cross-core sync).

Key optimizations:
  * EVERY matmul runs in the PE's 128x128 array mode, so the array never
    reconfigures (mode switches drain the systolic pipe and cost ~160ns
    per matmul when 64-row score MMs alternate with 128-row attnV MMs).
    Scores have K=dk=64 only, so kT is stored as TWO zero-padded
    stationaries: kT_E (even head rows 0:64, zeros below) and kT_O
    (zeros above, odd head rows 64:128); the full-height qT moving
    operand hits zeros for the other head's rows.
  * exp(scores) is split across TWO engines: ACT computes exact Exp;
    DVE computes a Schraudolph bit-trick exp in ONE tensor_scalar
    (i16 = A16*s + B16, int16 bits reinterpreted as bf16). Softmax
    renormalization cancels the ~3% multiplicative error.
  * Attention runs qh-sequentially (one 512-query half per pass) so only
    two 1-bank PSUM accumulators are live, freeing banks for a 5-deep
    score-tile pipeline (PE never waits on the exp engines).
  * All biases are folded off the PE: Q/K via the ACT evacuation
    (per-partition bias), V/O via scalar_tensor_tensor evacuations
    against pre-broadcast bias rows.
  * Softmax denominators ride along as a 65th 'ones' column of V.

Per-core device layout:
  xqT (512,1024) xkT/xvT (512,2048)  inputs transposed (dmodel on partitions)
  qT (128,1024)x4                    head h at chunk h//2, partitions 64*(h%2)
  kT_E/kT_O (128,2048)x4             zero-padded per head parity
  V  (128, 544)x16                   kv chunk tiles; head h cols [68h,68h+64)
                                     data, col 68h+64 = ones (denominator)
  ss (128,512) PSUM x5               scores [kv, q-half] (1 bank each)
  xs (65,512) PSUM x3                attnV accum; row 64 = softmax denom
"""
import numpy as np
import ml_dtypes

import concourse.bacc as bacc
import concourse.bass as bass
import concourse.mybir as mybir
import concourse.tile as tile
from concourse.bass_utils import run_bass_kernel_spmd

F32 = mybir.dt.float32
BF16 = mybir.dt.bfloat16
I16 = mybir.dt.int16
AF = mybir.ActivationFunctionType
ALU = mybir.AluOpType

B, L, D = 4, 2048, 512
H, DK = 8, 64
N_CORES = 8
LQ = L // 2            # query rows per core
P = 128
KVC = L // P           # 16 kv chunks
MC = D // P            # 4 dmodel chunks
VW = 68                # per-head stride in V tiles (64 data + ones + pad)
MASK_BIAS = np.float32(-30.0)   # large enough: exp(-30+s) ~ 0

# Schraudolph exp constants (bf16-bits variant): bf16_bits(exp(x)) ~=
# int16(A16*x + B16)
_A = 2.0 ** 23 / np.log(2.0)
_C = 486411.0
A16 = float(_A / 65536.0)
B16 = float((127.0 * 2.0 ** 23 - _C) / 65536.0)

MM_DT = BF16
MM_NP = ml_dtypes.bfloat16

_cache = {}


def _build():
    nc = bacc.Bacc("TRN2", target_bir_lowering=False, debug=False,
                   num_devices=N_CORES)

    xqT_d = nc.dram_tensor("xqT", [D, LQ], MM_DT, kind="ExternalInput").ap()
    xkT_d = nc.dram_tensor("xkT", [D, L], MM_DT, kind="ExternalInput").ap()
    xvT_d = nc.dram_tensor("xvT", [D, L], MM_DT, kind="ExternalInput").ap()
    wq_d = nc.dram_tensor("wq", [D, D], MM_DT, kind="ExternalInput").ap()
    wk_d = nc.dram_tensor("wk", [D, D], MM_DT, kind="ExternalInput").ap()
    wv_d = nc.dram_tensor("wv", [D, D], MM_DT, kind="ExternalInput").ap()
    wo_d = nc.dram_tensor("wo", [D, D], MM_DT, kind="ExternalInput").ap()
    bq_d = nc.dram_tensor("bq", [P, MC], F32, kind="ExternalInput").ap()
    bk_d = nc.dram_tensor("bk", [P, MC], F32, kind="ExternalInput").ap()
    bv_d = nc.dram_tensor("bv", [1, D], MM_DT, kind="ExternalInput").ap()
    bo_d = nc.dram_tensor("bo", [1, D], F32, kind="ExternalInput").ap()
    mb_d = nc.dram_tensor("mb", [P, KVC], F32, kind="ExternalInput").ap()
    b2_d = nc.dram_tensor("b2", [P, KVC], F32, kind="ExternalInput").ap()
    out_d = nc.dram_tensor("out", [LQ, D], F32, kind="ExternalOutput").ap()

    mm = nc.tensor.matmul

    with tile.TileContext(nc) as tc:
        with tc.tile_pool(name="const", bufs=1) as cpool, \
             tc.tile_pool(name="xin", bufs=1) as xpool, \
             tc.tile_pool(name="proj", bufs=1) as prpool, \
             tc.tile_pool(name="attn", bufs=8) as apool, \
             tc.tile_pool(name="norm", bufs=2) as npool, \
             tc.tile_pool(name="outp", bufs=3) as opool, \
             tc.tile_pool(name="ps", bufs=1, space="PSUM") as ps:

            def alloc_chunks(pool, ap2d, nm):
                return [pool.tile([P, ap2d.shape[1]], ap2d.dtype,
                                  tag=f"{nm}{kc}", name=f"{nm}{kc}")
                        for kc in range(MC)]

            def load_cols(tiles, ap2d, lo, hi):
                for kc in range(MC):
                    nc.sync.dma_start(tiles[kc][:, lo:hi],
                                      ap2d[kc * P:(kc + 1) * P, lo:hi])

            # loads in first-use order, 512-col granularity for inputs
            wq = alloc_chunks(cpool, wq_d, "wq")
            load_cols(wq, wq_d, 0, D)
            xqT = alloc_chunks(xpool, xqT_d, "xq")
            load_cols(xqT, xqT_d, 0, 512)
            bq = cpool.tile_from(bq_d)
            load_cols(xqT, xqT_d, 512, 1024)
            wk = alloc_chunks(cpool, wk_d, "wk")
            load_cols(wk, wk_d, 0, D)
            xkT = alloc_chunks(xpool, xkT_d, "xk")
            bk = cpool.tile_from(bk_d)
            for piece in range(4):
                load_cols(xkT, xkT_d, piece * 512, (piece + 1) * 512)
            wv = alloc_chunks(cpool, wv_d, "wv")
            load_cols(wv, wv_d, 0, D)
            xvT = alloc_chunks(xpool, xvT_d, "xv")
            bv = cpool.tile_from(bv_d)
            for piece in range(4):
                load_cols(xvT, xvT_d, piece * 512, (piece + 1) * 512)
            wo = alloc_chunks(cpool, wo_d, "wo")
            load_cols(wo, wo_d, 0, D)
            bo = cpool.tile_from(bo_d)
            mb = cpool.tile_from(mb_d)
            b2 = cpool.tile_from(b2_d)
            bv_bc = cpool.tile([P, D], MM_DT)
            nc.gpsimd.partition_broadcast(bv_bc[:], bv[:])
            bo_bc = cpool.tile([P, D], F32)
            nc.gpsimd.partition_broadcast(bo_bc[:], bo[:])

            def ss_tile(name):
                return ps.tile([P, 512], F32, tag="ss", bufs=5, name=name)

            # ---- Q projection (4 MMs + 1 ACT bias/evac per [128,512]) ----
            qT = [prpool.tile([P, LQ], MM_DT, tag=f"qT{m}", name=f"qT{m}")
                  for m in range(MC)]
            for m in range(MC):
                for s in range(LQ // 512):
                    pp = ss_tile("ppq")
                    for kc in range(MC):
                        mm(pp[:], wq[kc][:, m * P:(m + 1) * P],
                           xqT[kc][:, s * 512:(s + 1) * 512],
                           start=kc == 0, stop=kc == MC - 1)
                    nc.scalar.activation(qT[m][:, s * 512:(s + 1) * 512],
                                         pp[:], AF.Identity,
                                         bias=bq[:, m:m + 1])

            # ---- K projection, zero-padded per head parity ----
            kT_E = [prpool.tile([P, L], MM_DT, tag=f"kTE{m}", name=f"kTE{m}")
                    for m in range(MC)]
            kT_O = [prpool.tile([P, L], MM_DT, tag=f"kTO{m}", name=f"kTO{m}")
                    for m in range(MC)]
            for m in range(MC):
                nc.vector.memset(kT_E[m][64:128, :], 0.0)
                nc.vector.memset(kT_O[m][0:64, :], 0.0)
            for m in range(MC):
                for s in range(L // 512):
                    pp = ss_tile("ppk")
                    for kc in range(MC):
                        mm(pp[:], wk[kc][:, m * P:(m + 1) * P],
                           xkT[kc][:, s * 512:(s + 1) * 512],
                           start=kc == 0, stop=kc == MC - 1)
                    sl = slice(s * 512, (s + 1) * 512)
                    nc.scalar.activation(kT_E[m][0:64, sl], pp[0:64, :],
                                         AF.Identity, bias=bk[0:64, m:m + 1])
                    nc.scalar.activation(kT_O[m][64:128, sl], pp[64:128, :],
                                         AF.Identity, bias=bk[64:128, m:m + 1])

            # ---- V projection (bias + evac fused in one DVE op) ----
            v_sb = prpool.tile([P, KVC * VW * H], MM_DT, tag="V", name="v_sb")
            v_g = v_sb.rearrange("p (t h d) -> p t h d", t=KVC, d=VW)
            nc.vector.memset(v_sb[:], 1.0)
            for t in range(KVC):
                pv = ss_tile("ppv")
                for kc in range(MC):
                    mm(pv[:], xvT[kc][:, t * P:(t + 1) * P], wv[kc][:, :],
                       start=kc == 0, stop=kc == MC - 1)
                nc.vector.scalar_tensor_tensor(
                    v_g[:, t, :, 0:64],
                    pv.rearrange("p (h d) -> p h d", d=64), 1.0,
                    bv_bc.rearrange("p (h d) -> p h d", d=64),
                    ALU.mult, ALU.add)

            def v_head(t, h):
                return v_g[:, t, h, 0:65]

            # ---- attention: qh-sequential passes, all MMs 128x128 mode ----
            xsT2 = [prpool.tile([P, LQ], MM_DT, tag=f"xs{hp}",
                                name=f"xsT2_{hp}")
                    for hp in range(MC)]

            def out_proj(qt):
                po = ss_tile("ppo")
                for hp in range(MC):
                    mm(po[:], xsT2[hp][:, qt * P:(qt + 1) * P],
                       wo[hp][:, :], start=hp == 0, stop=hp == MC - 1)
                osb = opool.tile([P, 512], F32, tag="osb")
                nc.vector.scalar_tensor_tensor(osb[:], po[:], 1.0, bo_bc[:],
                                               ALU.mult, ALU.add)
                nc.sync.dma_start(out_d[qt * P:(qt + 1) * P, :], osb[:])

            for hp in range(MC):
                hE, hO = 2 * hp, 2 * hp + 1
                for qh in range(2):
                    xsE = ps.tile([65, 512], F32, tag="xs", bufs=3,
                                  name=f"xsE{hp}{qh}")
                    xsO = ps.tile([65, 512], F32, tag="xs", bufs=3,
                                  name=f"xsO{hp}{qh}")
                    at_tiles = {}
                    qsl = slice(qh * 512, (qh + 1) * 512)

                    def scores(c):
                        ssA = ss_tile("ssA")
                        ssB = ss_tile("ssB")
                        mm(ssA[:], kT_E[hp][:, c * P:(c + 1) * P],
                           qT[hp][:, qsl], start=True, stop=True)
                        mm(ssB[:], kT_O[hp][:, c * P:(c + 1) * P],
                           qT[hp][:, qsl], start=True, stop=True)
                        aE = apool.tile([P, 512], MM_DT, tag="atE", name="aE")
                        aO = apool.tile([P, 512], MM_DT, tag="atO", name="aO")
                        nc.scalar.activation(aE[:], ssA[:], AF.Exp,
                                             bias=mb[:, c:c + 1], scale=0.125)
                        if c == 7:   # small share back to ACT for balance
                            nc.scalar.activation(aO[:], ssB[:], AF.Exp,
                                                 bias=mb[:, c:c + 1],
                                                 scale=0.125)
                        else:
                            nc.vector.tensor_scalar(
                                aO.bitcast(I16)[:], ssB[:], A16 * 0.125,
                                b2[:, c:c + 1], ALU.mult, ALU.add)
                        at_tiles[c] = (aE, aO)

                    def attnv(c):
                        aE, aO = at_tiles.pop(c)
                        mm(xsE[:], v_head(c, hE), aE[:],
                           start=c == 0, stop=c == KVC - 1,
                           skip_group_check=True)
                        mm(xsO[:], v_head(c, hO), aO[:],
                           start=c == 0, stop=c == KVC - 1,
                           skip_group_check=True)

                    scores(0)
                    scores(1)
                    for c in range(2, KVC):
                        scores(c)
                        attnv(c - 2)
                    attnv(KVC - 2)
                    attnv(KVC - 1)

                    # normalize: xsT2 = xs[0:64] / xs[64] (denominator row)
                    for par, xs_t in ((0, xsE), (1, xsO)):
                        srow = npool.tile([1, 512], F32, tag="srow")
                        nc.scalar.copy(srow[:], xs_t[64:65, :])
                        rec = npool.tile([1, 512], F32, tag="rec")
                        nc.vector.reciprocal_approx_fast(rec[:], srow[:])
                        bc = npool.tile([64, 512], F32, tag="bc")
                        nc.gpsimd.partition_broadcast(bc[:], rec[:])
                        nc.vector.tensor_tensor(
                            xsT2[hp][64 * par:64 * par + 64, qsl],
                            xs_t[0:64, :], bc[:], ALU.mult)

                    # overlap the output projection with the last pass
                    if hp == MC - 1:
                        for qt in range(qh * 4, qh * 4 + 4):
                            out_proj(qt)

    nc.compile()
    return nc


def _host_inputs(query, key, value, mask, Wq, bq, Wk, bk, Wv, bv, Wo, bo):
    """Build the 8 per-core input maps (all rank-dependence lives here)."""
    f32 = np.float32
    wq_ = np.ascontiguousarray(Wq).astype(MM_NP)
    wk_ = np.ascontiguousarray(Wk).astype(MM_NP)
    wv_ = np.ascontiguousarray(Wv).astype(MM_NP)
    wo_ = np.ascontiguousarray(Wo).astype(MM_NP)
    bq_ = np.ascontiguousarray(bq.astype(f32).reshape(MC, P).T)
    bk_ = np.ascontiguousarray(bk.astype(f32).reshape(MC, P).T)
    bv_ = bv.astype(MM_NP).reshape(1, D)
    bo_ = bo.astype(f32).reshape(1, D)
    in_maps = []
    for c in range(N_CORES):
        b, half = c // 2, c % 2
        sl = slice(half * LQ, (half + 1) * LQ)
        xqT = np.ascontiguousarray(query[b, sl, :].T).astype(MM_NP)
        xkT = np.ascontiguousarray(key[b].T).astype(MM_NP)
        xvT = np.ascontiguousarray(value[b].T).astype(MM_NP)
        mbias = np.where(mask[b] == 0, MASK_BIAS, f32(0.0)).astype(f32)
        mb_ = np.ascontiguousarray(mbias.reshape(KVC, P).T)
        b2_ = (mb_ * f32(A16) + f32(B16)).astype(f32)
        in_maps.append({
            "xqT": xqT, "xkT": xkT, "xvT": xvT,
            "wq": wq_, "wk": wk_, "wv": wv_, "wo": wo_,
            "bq": bq_, "bk": bk_, "bv": bv_, "bo": bo_,
            "mb": mb_, "b2": b2_,
        })
    return in_maps


def kernel(query, key, value, mask, Wq, bq, Wk, bk, Wv, bv, Wo, bo):
    if "nc" not in _cache:
        _cache["nc"] = _build()
    nc = _cache["nc"]
    in_maps = _host_inputs(query, key, value, mask,
                           Wq, bq, Wk, bk, Wv, bv, Wo, bo)
    res = run_bass_kernel_spmd(nc, in_maps, list(range(N_CORES))).results
    out = np.empty((B, L, D), np.float32)
    for c in range(N_CORES):
        b, half = c // 2, c % 2
        out[b, half * LQ:(half + 1) * LQ, :] = res[c]["out"]
    return out


# revision 23
# speedup vs baseline: 1.6399x; 1.0322x over previous
"""Multi-head attention (B=4, L=2048, D=512, H=8) on 8 Trainium2 cores.

Sharding: core c handles batch b = c//2, query rows [(c%2)*1024, +1024).
Fully local: each core projects the FULL K/V for its batch (no collectives,
no cross-core sync).

Key optimizations:
  * EVERY matmul runs in the PE's 128x128 array mode, so the array never
    reconfigures (mode switches drain the systolic pipe and cost ~160ns
    per matmul when 64-row score MMs alternate with 128-row attnV MMs).
    Scores have K=dk=64 only, so kT is stored as TWO zero-padded
    stationaries: kT_E (even head rows 0:64, zeros below) and kT_O
    (zeros above, odd head rows 64:128); the full-height qT moving
    operand hits zeros for the other head's rows.
  * exp(scores) is split across TWO engines: ACT computes exact Exp;
    DVE computes a Schraudolph bit-trick exp in ONE tensor_scalar
    (i16 = A16*s + B16, int16 bits reinterpreted as bf16). Softmax
    renormalization cancels the ~3% multiplicative error.
  * Attention runs qh-sequentially (one 512-query half per pass) so only
    two 1-bank PSUM accumulators are live, freeing banks for a 5-deep
    score-tile pipeline (PE never waits on the exp engines).
  * All biases are folded off the PE: Q/K via the ACT evacuation
    (per-partition bias), V/O via scalar_tensor_tensor evacuations
    against pre-broadcast bias rows.
  * Softmax denominators ride along as a 65th 'ones' column of V.

Per-core device layout:
  xqT (512,1024) xkT/xvT (512,2048)  inputs transposed (dmodel on partitions)
  qT (128,1024)x4                    head h at chunk h//2, partitions 64*(h%2)
  kT_E/kT_O (128,2048)x4             zero-padded per head parity
  V  (128, 544)x16                   kv chunk tiles; head h cols [68h,68h+64)
                                     data, col 68h+64 = ones (denominator)
  ss (128,512) PSUM x5               scores [kv, q-half] (1 bank each)
  xs (65,512) PSUM x3                attnV accum; row 64 = softmax denom
"""
import numpy as np
import ml_dtypes

import concourse.bacc as bacc
import concourse.bass as bass
import concourse.mybir as mybir
import concourse.tile as tile
from concourse.bass_utils import run_bass_kernel_spmd

F32 = mybir.dt.float32
BF16 = mybir.dt.bfloat16
I16 = mybir.dt.int16
AF = mybir.ActivationFunctionType
ALU = mybir.AluOpType

B, L, D = 4, 2048, 512
H, DK = 8, 64
N_CORES = 8
LQ = L // 2            # query rows per core
P = 128
KVC = L // P           # 16 kv chunks
MC = D // P            # 4 dmodel chunks
VW = 68                # per-head stride in V tiles (64 data + ones + pad)
MASK_BIAS = np.float32(-30.0)   # large enough: exp(-30+s) ~ 0

# Schraudolph exp constants (bf16-bits variant): bf16_bits(exp(x)) ~=
# int16(A16*x + B16)
_A = 2.0 ** 23 / np.log(2.0)
_C = 486411.0
A16 = float(_A / 65536.0)
B16 = float((127.0 * 2.0 ** 23 - _C) / 65536.0)

MM_DT = BF16
MM_NP = ml_dtypes.bfloat16

_cache = {}


def _build():
    nc = bacc.Bacc("TRN2", target_bir_lowering=False, debug=False,
                   num_devices=N_CORES)

    # Inputs/weights are host-packed so every DMA moves whole 4KB DRAM rows:
    # piece tensors [n, 128, 4*512] hold one 512-col slice of all 4 chunks.
    xq_d = nc.dram_tensor("xq", [2, P, 2048], MM_DT, kind="ExternalInput").ap()
    xk_d = nc.dram_tensor("xk", [4, P, 2048], MM_DT, kind="ExternalInput").ap()
    xv_d = nc.dram_tensor("xv", [4, P, 2048], MM_DT, kind="ExternalInput").ap()
    wq_d = nc.dram_tensor("wq", [P, 2048], MM_DT, kind="ExternalInput").ap()
    wk_d = nc.dram_tensor("wk", [P, 2048], MM_DT, kind="ExternalInput").ap()
    wv_d = nc.dram_tensor("wv", [P, 2048], MM_DT, kind="ExternalInput").ap()
    wo_d = nc.dram_tensor("wo", [P, 2048], MM_DT, kind="ExternalInput").ap()
    bq_d = nc.dram_tensor("bq", [P, MC], F32, kind="ExternalInput").ap()
    bk_d = nc.dram_tensor("bk", [P, MC], F32, kind="ExternalInput").ap()
    bv_d = nc.dram_tensor("bv", [1, D], MM_DT, kind="ExternalInput").ap()
    bo_d = nc.dram_tensor("bo", [1, D], F32, kind="ExternalInput").ap()
    mb_d = nc.dram_tensor("mb", [P, KVC], F32, kind="ExternalInput").ap()
    b2_d = nc.dram_tensor("b2", [P, KVC], F32, kind="ExternalInput").ap()
    out_d = nc.dram_tensor("out", [LQ, D], F32, kind="ExternalOutput").ap()

    mm = nc.tensor.matmul

    with tile.TileContext(nc) as tc:
        with tc.tile_pool(name="const", bufs=1) as cpool, \
             tc.tile_pool(name="xin", bufs=1) as xpool, \
             tc.tile_pool(name="proj", bufs=1) as prpool, \
             tc.tile_pool(name="attn", bufs=8) as apool, \
             tc.tile_pool(name="norm", bufs=2) as npool, \
             tc.tile_pool(name="outp", bufs=3) as opool, \
             tc.tile_pool(name="ps", bufs=1, space="PSUM") as ps:

            def load_packed(pool, ap3d, nm, n):
                # one whole-tile DMA per piece: 4KB contiguous DRAM rows
                tiles = []
                for p_ in range(n):
                    t = pool.tile([P, 2048], MM_DT, tag=f"{nm}{p_}",
                                  name=f"{nm}{p_}")
                    nc.sync.dma_start(t[:], ap3d[p_])
                    tiles.append(t)
                return tiles

            def load_w(pool, ap2d, nm):
                t = pool.tile([P, 2048], MM_DT, tag=nm, name=nm)
                nc.sync.dma_start(t[:], ap2d[:, :])
                return t

            # loads in first-use order; piece tile p_, chunk kc lives at
            # columns [kc*512, +512) (weights: [kc*512 + j])
            wq = load_w(cpool, wq_d, "wq")
            xqP = load_packed(xpool, xq_d, "xq", 2)
            bq = cpool.tile_from(bq_d)
            wk = load_w(cpool, wk_d, "wk")
            bk = cpool.tile_from(bk_d)
            xkP = load_packed(xpool, xk_d, "xk", 4)
            wv = load_w(cpool, wv_d, "wv")
            bv = cpool.tile_from(bv_d)
            xvP = load_packed(xpool, xv_d, "xv", 4)
            wo = load_w(cpool, wo_d, "wo")
            bo = cpool.tile_from(bo_d)
            mb = cpool.tile_from(mb_d)
            b2 = cpool.tile_from(b2_d)

            def w_chunk(w, kc, lo, hi):
                return w[:, kc * 512 + lo:kc * 512 + hi]
            bv_bc = cpool.tile([P, D], MM_DT)
            nc.gpsimd.partition_broadcast(bv_bc[:], bv[:])
            bo_bc = cpool.tile([P, D], F32)
            nc.gpsimd.partition_broadcast(bo_bc[:], bo[:])

            def ss_tile(name):
                return ps.tile([P, 512], F32, tag="ss", bufs=5, name=name)

            # ---- Q projection (4 MMs + 1 ACT bias/evac per [128,512]) ----
            qT = [prpool.tile([P, LQ], MM_DT, tag=f"qT{m}", name=f"qT{m}")
                  for m in range(MC)]
            for m in range(MC):
                for s in range(LQ // 512):
                    pp = ss_tile("ppq")
                    for kc in range(MC):
                        mm(pp[:], w_chunk(wq, kc, m * P, (m + 1) * P),
                           xqP[s][:, kc * 512:(kc + 1) * 512],
                           start=kc == 0, stop=kc == MC - 1)
                    nc.scalar.activation(qT[m][:, s * 512:(s + 1) * 512],
                                         pp[:], AF.Identity,
                                         bias=bq[:, m:m + 1])

            # ---- K projection, zero-padded per head parity ----
            kT_E = [prpool.tile([P, L], MM_DT, tag=f"kTE{m}", name=f"kTE{m}")
                    for m in range(MC)]
            kT_O = [prpool.tile([P, L], MM_DT, tag=f"kTO{m}", name=f"kTO{m}")
                    for m in range(MC)]
            for m in range(MC):
                nc.vector.memset(kT_E[m][64:128, :], 0.0)
                nc.vector.memset(kT_O[m][0:64, :], 0.0)
            for m in range(MC):
                for s in range(L // 512):
                    pp = ss_tile("ppk")
                    for kc in range(MC):
                        mm(pp[:], w_chunk(wk, kc, m * P, (m + 1) * P),
                           xkP[s][:, kc * 512:(kc + 1) * 512],
                           start=kc == 0, stop=kc == MC - 1)
                    sl = slice(s * 512, (s + 1) * 512)
                    nc.scalar.activation(kT_E[m][0:64, sl], pp[0:64, :],
                                         AF.Identity, bias=bk[0:64, m:m + 1])
                    nc.scalar.activation(kT_O[m][64:128, sl], pp[64:128, :],
                                         AF.Identity, bias=bk[64:128, m:m + 1])

            # ---- V projection (bias + evac fused in one DVE op) ----
            v_sb = prpool.tile([P, KVC * VW * H], MM_DT, tag="V", name="v_sb")
            v_g = v_sb.rearrange("p (t h d) -> p t h d", t=KVC, d=VW)
            nc.vector.memset(v_sb[:], 1.0)
            for t in range(KVC):
                pv = ss_tile("ppv")
                for kc in range(MC):
                    mm(pv[:], xvP[t // 4][:, kc * 512 + (t % 4) * P:
                                          kc * 512 + (t % 4 + 1) * P],
                       w_chunk(wv, kc, 0, 512),
                       start=kc == 0, stop=kc == MC - 1)
                nc.vector.scalar_tensor_tensor(
                    v_g[:, t, :, 0:64],
                    pv.rearrange("p (h d) -> p h d", d=64), 1.0,
                    bv_bc.rearrange("p (h d) -> p h d", d=64),
                    ALU.mult, ALU.add)

            def v_head(t, h):
                return v_g[:, t, h, 0:65]

            # ---- attention: qh-sequential passes, all MMs 128x128 mode ----
            xsT2 = [prpool.tile([P, LQ], MM_DT, tag=f"xs{hp}",
                                name=f"xsT2_{hp}")
                    for hp in range(MC)]

            def out_proj(qt):
                po = ss_tile("ppo")
                for hp in range(MC):
                    mm(po[:], xsT2[hp][:, qt * P:(qt + 1) * P],
                       w_chunk(wo, hp, 0, 512), start=hp == 0,
                       stop=hp == MC - 1)
                osb = opool.tile([P, 512], F32, tag="osb")
                nc.vector.scalar_tensor_tensor(osb[:], po[:], 1.0, bo_bc[:],
                                               ALU.mult, ALU.add)
                nc.sync.dma_start(out_d[qt * P:(qt + 1) * P, :], osb[:])

            for hp in range(MC):
                hE, hO = 2 * hp, 2 * hp + 1
                for qh in range(2):
                    xsE = ps.tile([65, 512], F32, tag="xs", bufs=3,
                                  name=f"xsE{hp}{qh}")
                    xsO = ps.tile([65, 512], F32, tag="xs", bufs=3,
                                  name=f"xsO{hp}{qh}")
                    at_tiles = {}
                    qsl = slice(qh * 512, (qh + 1) * 512)

                    def scores(c):
                        ssA = ss_tile("ssA")
                        ssB = ss_tile("ssB")
                        mm(ssA[:], kT_E[hp][:, c * P:(c + 1) * P],
                           qT[hp][:, qsl], start=True, stop=True)
                        mm(ssB[:], kT_O[hp][:, c * P:(c + 1) * P],
                           qT[hp][:, qsl], start=True, stop=True)
                        aE = apool.tile([P, 512], MM_DT, tag="atE", name="aE")
                        aO = apool.tile([P, 512], MM_DT, tag="atO", name="aO")
                        nc.scalar.activation(aE[:], ssA[:], AF.Exp,
                                             bias=mb[:, c:c + 1], scale=0.125)
                        if c in (5, 11):   # share back to ACT for balance
                            nc.scalar.activation(aO[:], ssB[:], AF.Exp,
                                                 bias=mb[:, c:c + 1],
                                                 scale=0.125)
                        else:
                            nc.vector.tensor_scalar(
                                aO.bitcast(I16)[:], ssB[:], A16 * 0.125,
                                b2[:, c:c + 1], ALU.mult, ALU.add)
                        at_tiles[c] = (aE, aO)

                    def attnv(c):
                        aE, aO = at_tiles.pop(c)
                        mm(xsE[:], v_head(c, hE), aE[:],
                           start=c == 0, stop=c == KVC - 1,
                           skip_group_check=True)
                        mm(xsO[:], v_head(c, hO), aO[:],
                           start=c == 0, stop=c == KVC - 1,
                           skip_group_check=True)

                    scores(0)
                    scores(1)
                    for c in range(2, KVC):
                        scores(c)
                        attnv(c - 2)
                    attnv(KVC - 2)
                    attnv(KVC - 1)

                    # normalize: xsT2 = xs[0:64] / xs[64] (denominator row)
                    # (approx-recip is a custom DVE op: stage the denominator
                    # row to SBUF first — reading PSUM there breaks on HW)
                    for par, xs_t in ((0, xsE), (1, xsO)):
                        srow = npool.tile([1, 512], F32, tag="srow")
                        nc.scalar.copy(srow[:], xs_t[64:65, :])
                        rec = npool.tile([1, 512], F32, tag="rec")
                        nc.vector.reciprocal_approx_fast(rec[:], srow[:])
                        bc = npool.tile([64, 512], F32, tag="bc")
                        nc.gpsimd.partition_broadcast(bc[:], rec[:])
                        nc.vector.tensor_tensor(
                            xsT2[hp][64 * par:64 * par + 64, qsl],
                            xs_t[0:64, :], bc[:], ALU.mult)

                    # overlap the output projection with the last pass
                    if hp == MC - 1:
                        for qt in range(qh * 4, qh * 4 + 4):
                            out_proj(qt)

    nc.compile()
    return nc


def _host_inputs(query, key, value, mask, Wq, bq, Wk, bk, Wv, bv, Wo, bo):
    """Build the 8 per-core input maps (all rank-dependence lives here)."""
    f32 = np.float32

    def pack_w(W):
        # [512, 512] -> [128, 2048]: chunk kc at columns [kc*512, +512)
        w = np.asarray(W).astype(MM_NP).reshape(MC, P, D)
        return np.ascontiguousarray(w.transpose(1, 0, 2).reshape(P, MC * D))

    def pack_x(x, n):
        # x [rows, 512] -> pieces [n, 128, 2048]: piece p_ holds the
        # 512-row slice p_ of x transposed, chunk kc at cols [kc*512,+512)
        xT = np.ascontiguousarray(x.T).astype(MM_NP)        # [512, rows]
        rows = xT.shape[1]
        pw = rows // n
        out = np.empty((n, P, MC * pw), MM_NP)
        for p_ in range(n):
            blk = xT[:, p_ * pw:(p_ + 1) * pw].reshape(MC, P, pw)
            out[p_] = blk.transpose(1, 0, 2).reshape(P, MC * pw)
        return out

    wq_ = pack_w(Wq)
    wk_ = pack_w(Wk)
    wv_ = pack_w(Wv)
    wo_ = pack_w(Wo)
    bq_ = np.ascontiguousarray(bq.astype(f32).reshape(MC, P).T)
    bk_ = np.ascontiguousarray(bk.astype(f32).reshape(MC, P).T)
    bv_ = bv.astype(MM_NP).reshape(1, D)
    bo_ = bo.astype(f32).reshape(1, D)
    in_maps = []
    for c in range(N_CORES):
        b, half = c // 2, c % 2
        sl = slice(half * LQ, (half + 1) * LQ)
        xq_ = pack_x(query[b, sl, :], 2)
        xk_ = pack_x(key[b], 4)
        xv_ = pack_x(value[b], 4)
        mbias = np.where(mask[b] == 0, MASK_BIAS, f32(0.0)).astype(f32)
        mb_ = np.ascontiguousarray(mbias.reshape(KVC, P).T)
        b2_ = (mb_ * f32(A16) + f32(B16)).astype(f32)
        in_maps.append({
            "xq": xq_, "xk": xk_, "xv": xv_,
            "wq": wq_, "wk": wk_, "wv": wv_, "wo": wo_,
            "bq": bq_, "bk": bk_, "bv": bv_, "bo": bo_,
            "mb": mb_, "b2": b2_,
        })
    return in_maps


def kernel(query, key, value, mask, Wq, bq, Wk, bk, Wv, bv, Wo, bo):
    if "nc" not in _cache:
        _cache["nc"] = _build()
    nc = _cache["nc"]
    in_maps = _host_inputs(query, key, value, mask,
                           Wq, bq, Wk, bk, Wv, bv, Wo, bo)
    res = run_bass_kernel_spmd(nc, in_maps, list(range(N_CORES))).results
    out = np.empty((B, L, D), np.float32)
    for c in range(N_CORES):
        b, half = c // 2, c % 2
        out[b, half * LQ:(half + 1) * LQ, :] = res[c]["out"]
    return out
